# revision 9
# baseline (speedup 1.0000x reference)
"""3-layer GAT (heads=1) on Trainium2, 8 NeuronCores, node-sharded.

Strategy (graph/data parallel per sharding hint):
  - Nodes are partitioned contiguously across 8 cores (12500 each).
  - Per layer:
      stage1 (sharded): H'[v] = [h_in[v] @ W | 1 | h@W@a_src | h@W@a_dst]
      AllGather H' shards -> every core holds full H' [100000, 68]
      stage2 (edges, dst-sharded): for each block of 128 local dst nodes,
        gather H'[src] rows (indirect DMA), compute per-edge
        w = exp(leaky_relu(a_s[src] + a_d[dst])), build one-hot*w selection
        matrix S_w [128e x 128d] and PSUM-accumulate S_w.T @ [h|1] over the
        block's edge tiles -> numerator (64) + softmax denominator (1).
        Divide, add bias, activation; store transposed for next stage1.
  - Edge lists are preprocessed on host: sorted by dst, cut into per-block
    tiles of 128 edges padded to a uniform tile count T per block.
"""

import os
import sys
import math

import numpy as np

for _p in ("/opt/trn_rl_repo", "/root/.axon_site/_ro/trn_rl_repo"):
    if os.path.isdir(_p) and _p not in sys.path:
        sys.path.insert(0, _p)

import concourse.bacc as bacc  # noqa: E402
import concourse.tile as tile  # noqa: E402
from concourse import bass, mybir  # noqa: E402
from concourse import bass_utils  # noqa: E402
from concourse.masks import make_identity  # noqa: E402

P = 128
NCORES = 8
IN_CH = 128
HID = 64
OUT_CH = 64
NEG_SLOPE = 0.2
HCOL = 68  # H' row: h(0:64) | ones(64) | a_src(65) | a_dst(66) | pad(67)
GCOL = 66  # gathered columns per src row: h + ones + a_src

F32 = mybir.dt.float32
I32 = mybir.dt.int32

_CACHE = {}
_last_in_maps = None


def _ceil_div(a, b):
    return (a + b - 1) // b


def _prep_edges(src, dst, n_nodes, nsh, nblk):
    """Sort edges by dst, bucket into per-core 128-dst blocks, pad each block
    to a uniform number T of 128-edge tiles. Returns per-core tables laid out
    [128, nblk*T] plus T."""
    order = np.argsort(dst, kind="stable")
    src_s = src[order].astype(np.int32)
    dst_s = dst[order].astype(np.int32)

    core = dst_s // nsh
    off = dst_s - core * nsh
    blk = off // P  # block within core
    gblk = core * nblk + blk

    nblk_tot = NCORES * nblk
    counts = np.bincount(gblk, minlength=nblk_tot)
    T = max(2, _ceil_div(int(counts.max()), P))

    # order edges by (block, src) so gathers walk HBM mostly forward
    order2 = np.lexsort((src_s, gblk))
    src_s = src_s[order2]
    dst_s = dst_s[order2]
    gblk = gblk[order2]
    off = off[order2]

    starts = np.zeros(nblk_tot + 1, np.int64)
    np.cumsum(counts, out=starts[1:])
    pos = np.arange(len(src_s)) - starts[gblk]
    slot = gblk.astype(np.int64) * (T * P) + pos

    tot = nblk_tot * T * P
    srcF = np.zeros(tot, np.int32)
    dofF = np.full(tot, 200.0, np.float32)
    dlocF = np.zeros(tot, np.int32)
    srcF[slot] = src_s
    dofF[slot] = (off - (off // P) * P).astype(np.float32)
    dlocF[slot] = off

    def split(a):
        # [core][128 partitions, nblk*T columns]
        return np.ascontiguousarray(
            a.reshape(NCORES, nblk * T, P).transpose(0, 2, 1)
        )

    return split(srcF), split(dofF), split(dlocF), T


def _build(n_nodes, T):
    nsh = n_nodes // NCORES
    nblk = _ceil_div(nsh, P)
    ncol = nblk * T

    nc = bacc.Bacc(
        "TRN2",
        target_bir_lowering=False,
        debug=False,
        enable_asserts=False,
        num_devices=NCORES,
    )

    xT = nc.dram_tensor("xT", [IN_CH, nsh], F32, kind="ExternalInput")
    srcT = nc.dram_tensor("srcT", [P, ncol], I32, kind="ExternalInput")
    dofT = nc.dram_tensor("dofT", [P, ncol], F32, kind="ExternalInput")
    dlocT = nc.dram_tensor("dlocT", [P, ncol], I32, kind="ExternalInput")
    waug_d = [
        nc.dram_tensor(f"waug{l}", [IN_CH if l == 0 else HID, HCOL], F32,
                       kind="ExternalInput")
        for l in range(3)
    ]
    bias_d = [
        nc.dram_tensor(f"bias{l}", [P, HID], F32, kind="ExternalInput")
        for l in range(3)
    ]
    out_d = nc.dram_tensor("out", [nsh, OUT_CH], F32, kind="ExternalOutput")

    with tile.TileContext(nc) as tc:
        with (
            tc.tile_pool(name="const", bufs=1) as cpool,
            tc.tile_pool(name="meta", bufs=3) as mpool,
            tc.tile_pool(name="gbuf", bufs=8) as gpool,
            tc.tile_pool(name="swbuf", bufs=8) as swpool,
            tc.tile_pool(name="stage", bufs=3) as stpool,
            tc.tile_pool(name="small", bufs=8) as smpool,
            tc.tile_pool(name="ps1", bufs=2, space="PSUM") as ps1,
            tc.tile_pool(name="psb", bufs=2, space="PSUM") as psb,
            tc.tile_pool(name="pst", bufs=2, space="PSUM") as pst,
            tc.tile_pool(name="dram", bufs=1, space="DRAM") as dpool,
        ):
            iota_i = cpool.tile([P, P], I32)
            nc.gpsimd.iota(iota_i[:], pattern=[[1, P]], base=0,
                           channel_multiplier=0)
            iota_f = cpool.tile([P, P], F32)
            nc.vector.tensor_copy(iota_f[:], iota_i[:])
            ident = cpool.tile([P, P], F32)
            make_identity(nc, ident[:])

            waug_t = []
            bias_t = []
            for l in range(3):
                cin = IN_CH if l == 0 else HID
                wt = cpool.tile([cin, HCOL], F32, tag=f"waug{l}")
                nc.sync.dma_start(out=wt[:], in_=waug_d[l].ap()[:, :])
                waug_t.append(wt)
                bt = cpool.tile([P, HID], F32, tag=f"bias{l}")
                nc.sync.dma_start(out=bt[:], in_=bias_d[l].ap()[:, :])
                bias_t.append(bt)

            hp_sh = [dpool.tile([nsh, HCOL], F32, tag=f"hpsh{l}", name=f"hpsh{l}")
                     for l in range(3)]
            hp_full = [dpool.tile([n_nodes, HCOL], F32, tag=f"hpfull{l}",
                                  addr_space="Shared", name=f"hpfull{l}")
                       for l in range(3)]
            h_inT = [None,
                     dpool.tile([HID, nsh], F32, tag="hint1", name="hint1"),
                     dpool.tile([HID, nsh], F32, tag="hint2", name="hint2")]

            for l in range(3):
                cin = IN_CH if l == 0 else HID
                # ---- stage 1: H'_shard = h_in @ Waug (+ ones col) ----
                for i in range(nblk):
                    r0 = i * P
                    rn = min(P, nsh - r0)
                    lh = stpool.tile([cin, P], F32, tag="lh")
                    if l == 0:
                        nc.sync.dma_start(out=lh[:, :rn],
                                          in_=xT.ap()[:, r0:r0 + rn])
                    else:
                        nc.sync.dma_start(out=lh[:, :rn],
                                          in_=h_inT[l][0:HID, r0:r0 + rn])
                    p1 = ps1.tile([P, HCOL], F32, tag="p1")
                    nc.tensor.matmul(out=p1[:rn, :], lhsT=lh[:, :rn],
                                     rhs=waug_t[l][:], start=True, stop=True)
                    stg = stpool.tile([P, HCOL], F32, tag="stg")
                    nc.scalar.activation(stg[:rn, :], p1[:rn, :],
                                         mybir.ActivationFunctionType.Copy)
                    nc.gpsimd.memset(stg[:rn, 64:65], 1.0)
                    nc.sync.dma_start(out=hp_sh[l][r0:r0 + rn, :],
                                      in_=stg[:rn, :])

                # ---- AllGather shards -> full H' ----
                nc.gpsimd.collective_compute(
                    "AllGather",
                    mybir.AluOpType.bypass,
                    replica_groups=[list(range(NCORES))],
                    ins=[hp_sh[l].opt()],
                    outs=[hp_full[l].opt()],
                )

                # ---- stage 2: per 128-dst block ----
                for b in range(nblk):
                    b0 = b * P
                    bn = min(P, nsh - b0)
                    c0 = b * T
                    srcs = mpool.tile([P, T], I32, tag="srcs")
                    nc.sync.dma_start(out=srcs[:], in_=srcT.ap()[:, c0:c0 + T])
                    doffs = mpool.tile([P, T], F32, tag="doffs")
                    nc.sync.dma_start(out=doffs[:], in_=dofT.ap()[:, c0:c0 + T])
                    dlocs = mpool.tile([P, T], I32, tag="dlocs")
                    nc.sync.dma_start(out=dlocs[:], in_=dlocT.ap()[:, c0:c0 + T])

                    pblk = psb.tile([P, GCOL - 1], F32, tag="pblk")
                    for j in range(T):
                        g = gpool.tile([P, GCOL], F32, tag="g")
                        nc.gpsimd.indirect_dma_start(
                            out=g[:],
                            out_offset=None,
                            in_=hp_full[l][:, :],
                            in_offset=bass.IndirectOffsetOnAxis(
                                ap=srcs[:, j:j + 1], axis=0),
                        )
                        a_d = smpool.tile([P, 1], F32, tag="a_d")
                        nc.gpsimd.indirect_dma_start(
                            out=a_d[:],
                            out_offset=None,
                            in_=hp_sh[l][:, :],
                            in_offset=bass.IndirectOffsetOnAxis(
                                ap=dlocs[:, j:j + 1], axis=0),
                            element_offset=66,
                        )
                        e = smpool.tile([P, 1], F32, tag="e")
                        nc.vector.tensor_tensor(out=e[:], in0=g[:, 65:66],
                                                in1=a_d[:, 0:1],
                                                op=mybir.AluOpType.add)
                        e2 = smpool.tile([P, 1], F32, tag="e2")
                        nc.scalar.mul(e2[:], e[:], NEG_SLOPE)
                        lre = smpool.tile([P, 1], F32, tag="lre")
                        nc.vector.tensor_tensor(out=lre[:], in0=e[:],
                                                in1=e2[:],
                                                op=mybir.AluOpType.max)
                        w = smpool.tile([P, 1], F32, tag="w")
                        nc.scalar.activation(w[:], lre[:],
                                             mybir.ActivationFunctionType.Exp)
                        sw = swpool.tile([P, P], F32, tag="sw")
                        nc.vector.tensor_scalar(
                            out=sw[:],
                            in0=iota_f[:],
                            scalar1=doffs[:, j:j + 1],
                            scalar2=w[:, 0:1],
                            op0=mybir.AluOpType.is_equal,
                            op1=mybir.AluOpType.mult,
                        )
                        nc.tensor.matmul(out=pblk[:bn, :],
                                         lhsT=sw[:, :bn],
                                         rhs=g[:, 0:65],
                                         start=(j == 0), stop=(j == T - 1))

                    rec = smpool.tile([P, 1], F32, tag="rec")
                    nc.vector.reciprocal(rec[:bn], pblk[:bn, 64:65])
                    ob = stpool.tile([P, HID], F32, tag="ob")
                    nc.vector.tensor_scalar(out=ob[:bn, :],
                                            in0=pblk[:bn, 0:64],
                                            scalar1=rec[:bn, 0:1],
                                            scalar2=None,
                                            op0=mybir.AluOpType.mult)
                    ob2 = stpool.tile([P, HID], F32, tag="ob2")
                    nc.vector.tensor_tensor(out=ob2[:bn, :], in0=ob[:bn, :],
                                            in1=bias_t[l][:bn, :],
                                            op=mybir.AluOpType.add)
                    act = stpool.tile([P, HID], F32, tag="act")
                    if l < 2:
                        nc.scalar.activation(act[:bn, :], ob2[:bn, :],
                                             mybir.ActivationFunctionType.Relu)
                        ptr = pst.tile([HID, P], F32, tag="ptr")
                        nc.tensor.transpose(out=ptr[:, :bn], in_=act[:bn, :],
                                            identity=ident[:bn, :bn])
                        hts = stpool.tile([HID, P], F32, tag="hts")
                        nc.vector.tensor_copy(hts[:, :bn], ptr[:, :bn])
                        nc.sync.dma_start(
                            out=h_inT[l + 1][0:HID, b0:b0 + bn],
                            in_=hts[:, :bn])
                    else:
                        nc.scalar.activation(
                            act[:bn, :], ob2[:bn, :],
                            mybir.ActivationFunctionType.Sigmoid)
                        nc.sync.dma_start(out=out_d.ap()[b0:b0 + bn, :],
                                          in_=act[:bn, :])

    nc.compile()
    return nc


def _run(x, edge_index, params, n_nodes, trace=False):
    nsh = n_nodes // NCORES
    nblk = _ceil_div(nsh, P)

    src = np.asarray(edge_index[0], np.int64)
    dst = np.asarray(edge_index[1], np.int64)
    loop = np.arange(n_nodes, dtype=np.int64)
    src = np.concatenate([src, loop])
    dst = np.concatenate([dst, loop])

    srcT, dofT, dlocT, T = _prep_edges(src, dst, n_nodes, nsh, nblk)

    key = (n_nodes, T)
    if key not in _CACHE:
        _CACHE[key] = _build(n_nodes, T)
    nc = _CACHE[key]

    xT = np.ascontiguousarray(np.asarray(x, np.float32).T)

    shared = {}
    for l in range(3):
        W = np.asarray(params[f"W{l}"], np.float32)
        cin = W.shape[0]
        waug = np.zeros((cin, HCOL), np.float32)
        waug[:, 0:64] = W
        waug[:, 65] = W @ np.asarray(params[f"a_src{l}"], np.float32)
        waug[:, 66] = W @ np.asarray(params[f"a_dst{l}"], np.float32)
        shared[f"waug{l}"] = waug
        shared[f"bias{l}"] = np.ascontiguousarray(
            np.broadcast_to(np.asarray(params[f"b{l}"], np.float32), (P, HID)))

    in_maps = []
    for c in range(NCORES):
        m = dict(shared)
        m["xT"] = np.ascontiguousarray(xT[:, c * nsh:(c + 1) * nsh])
        m["srcT"] = srcT[c]
        m["dofT"] = dofT[c]
        m["dlocT"] = dlocT[c]
        in_maps.append(m)

    global _last_in_maps
    _last_in_maps = in_maps
    res = bass_utils.run_bass_kernel_spmd(
        nc, in_maps, core_ids=list(range(NCORES)), trace=trace)
    out = np.concatenate([res.results[c]["out"] for c in range(NCORES)], axis=0)
    if trace:
        return out, res
    return out


def kernel(**inputs):
    x = inputs["x"]
    edge_index = inputs["edge_index"]
    return _run(x, edge_index, inputs, x.shape[0])
